# revision 1
# baseline (speedup 1.0000x reference)
"""Gated cosine-affinity kernel for Trainium2 (Bass/Tile), 8-core SPMD.

Problem: for each batch b (B=8):
    Xg = A_1 * X;  Yg = A_2 * Y            (elementwise gates)
    out[b] = normalize_rows(Xg) @ normalize_rows(Yg).T      (2048 x 2048)
with row norm = sqrt(max(|row|^2, 1e-6)).

Sharding: data-parallel over batch — one batch element per NeuronCore.
Per-core pipeline:
  stage 1: gate X/Y, compute row norms (ACT Square+accum, Newton-refined
           sqrt), normalize Y, PE-transpose both into d-major layout.
  stage 2: 16x4 matmuls (d=128 contraction on partitions), fold X's
           1/norm into the PSUM->SBUF evacuation as a per-partition
           scale, stream 1 MiB row-block stores to DRAM.

The kernel is memory-bound: ~21 MB of HBM traffic per core (4 MiB loads
+ 16 MiB stores) against ~360 GB/s per-core HBM bandwidth.
"""

import numpy as np
from contextlib import ExitStack

import concourse.bass as bass
import concourse.tile as tile
from concourse import bacc, mybir
from concourse.bass_utils import run_bass_kernel_spmd
from concourse.masks import make_identity

B = 8
N = 2048          # rows of X (output rows)
M = 2048          # rows of Y (output cols)
D = 128           # feature dim == partition count == contraction dim
P = 128
EPS = 1e-6
NCH = N // P      # 16 row-chunks per tensor
MM_N = 512        # matmul moving free dim (one PSUM bank of fp32)
NMM = M // MM_N   # 4 matmuls per output row-block

FP32 = mybir.dt.float32
AF = mybir.ActivationFunctionType

_CACHED_NC = None


def _build_program():
    nc = bacc.Bacc("TRN2", target_bir_lowering=False, debug=False, num_devices=B)

    Xd = nc.dram_tensor("X", [N, D], FP32, kind="ExternalInput")
    Yd = nc.dram_tensor("Y", [M, D], FP32, kind="ExternalInput")
    A1d = nc.dram_tensor("A_1", [N, D], FP32, kind="ExternalInput")
    A2d = nc.dram_tensor("A_2", [M, D], FP32, kind="ExternalInput")
    OUT = nc.dram_tensor("out", [N, M], FP32, kind="ExternalOutput")

    with tile.TileContext(nc) as tc, ExitStack() as ctx:
        consts = ctx.enter_context(tc.tile_pool(name="consts", bufs=1))
        raw = ctx.enter_context(tc.tile_pool(name="raw", bufs=1))
        gated = ctx.enter_context(tc.tile_pool(name="gated", bufs=1))
        small = ctx.enter_context(tc.tile_pool(name="small", bufs=1))
        scratch = ctx.enter_context(tc.tile_pool(name="scratch", bufs=2))
        yn_pool = ctx.enter_context(tc.tile_pool(name="yn", bufs=4))
        tmat = ctx.enter_context(tc.tile_pool(name="tmat", bufs=1))
        ob_pool = ctx.enter_context(tc.tile_pool(name="ob", bufs=3))
        psum_t = ctx.enter_context(tc.tile_pool(name="psum_t", bufs=3, space="PSUM"))
        psum_mm = ctx.enter_context(tc.tile_pool(name="psum_mm", bufs=5, space="PSUM"))

        ident = consts.tile([P, P], FP32)
        make_identity(nc, ident)

        # Alternate PSUM evacuations between VectorE and ScalarE so neither
        # engine becomes the bottleneck (both are ~0.6us per [128,512] tile).
        copy_state = {"i": 0}

        def evac(dst, src, scale=None):
            use_vector = copy_state["i"] % 2 == 0
            copy_state["i"] += 1
            if scale is None:
                if use_vector:
                    nc.vector.tensor_copy(dst, src)
                else:
                    nc.scalar.copy(dst, src)
            else:
                if use_vector:
                    nc.vector.tensor_scalar_mul(dst, src, scale)
                else:
                    nc.scalar.mul(dst, src, scale)

        def rownorm_inv(sums, name):
            """inv = 1/sqrt(max(sums, EPS)), Newton-refined against ACT's
            low-precision Sqrt table. All ops on [128, NCH] — negligible."""
            v = small.tile([P, NCH], FP32, tag=f"{name}_v")
            s = small.tile([P, NCH], FP32, tag=f"{name}_s")
            r = small.tile([P, NCH], FP32, tag=f"{name}_r")
            t = small.tile([P, NCH], FP32, tag=f"{name}_t")
            inv = small.tile([P, NCH], FP32, tag=f"{name}_inv")
            nc.vector.tensor_scalar_max(v, sums, EPS)
            nc.scalar.sqrt(s, v)
            nc.vector.reciprocal(r, s)
            nc.vector.tensor_mul(t, v, r)          # t = v/s
            nc.vector.tensor_add(t, t, s)          # t = s + v/s
            nc.vector.tensor_scalar_mul(t, t, 0.5)  # Newton: sqrt(v) to ~fp32
            nc.vector.reciprocal(inv, t)
            return inv

        def load_and_gate(Td, Ad, name):
            """Load T and its gate, multiply, and accumulate row sum-squares
            (ACT Square w/ accum_out). Returns (gated_tile, sums_tile)."""
            Tv = Td.rearrange("(c p) d -> p c d", p=P)
            Av = Ad.rearrange("(c p) d -> p c d", p=P)
            traw = raw.tile([P, NCH, D], FP32, tag=f"{name}_raw")
            araw = raw.tile([P, NCH, D], FP32, tag=f"{name}_araw")
            nc.sync.dma_start(out=traw, in_=Tv)
            nc.sync.dma_start(out=araw, in_=Av)
            g = gated.tile([P, NCH, D], FP32, tag=f"{name}_g")
            sums = small.tile([P, NCH], FP32, tag=f"{name}_sums")
            for c in range(NCH):
                nc.vector.tensor_mul(g[:, c, :], traw[:, c, :], araw[:, c, :])
                sq = scratch.tile([P, D], FP32, tag="sq")
                nc.scalar.activation(
                    sq, g[:, c, :], AF.Square, accum_out=sums[:, c : c + 1]
                )
            return g, sums

        # ---- stage 1: Y (normalized pre-transpose; its norm scales output
        # columns, which must be applied before the matmul) ----
        yg, ysums = load_and_gate(Yd, A2d, "y")
        yinv = rownorm_inv(ysums, "y")
        YnT = tmat.tile([P, M], FP32, tag="YnT")
        for c in range(NCH):
            yn = yn_pool.tile([P, D], FP32, tag="yn")
            nc.vector.tensor_scalar_mul(yn, yg[:, c, :], yinv[:, c : c + 1])
            pt = psum_t.tile([P, P], FP32, tag="pt")
            nc.tensor.transpose(pt, yn, ident)
            evac(YnT[:, c * P : (c + 1) * P], pt)

        # ---- stage 1: X (transposed unnormalized; 1/norm is folded into the
        # stage-2 PSUM evacuation as a per-partition scale) ----
        xg, xsums = load_and_gate(Xd, A1d, "x")
        XgT = tmat.tile([P, N], FP32, tag="XgT")
        for c in range(NCH):
            pt = psum_t.tile([P, P], FP32, tag="pt")
            nc.tensor.transpose(pt, xg[:, c, :], ident)
            evac(XgT[:, c * P : (c + 1) * P], pt)
        xinv = rownorm_inv(xsums, "x")

        # ---- stage 2: row-block matmuls + scaled evacuation + 1 MiB stores ----
        for n in range(NCH):
            ob = ob_pool.tile([P, M], FP32, tag="ob")
            for m in range(NMM):
                pm = psum_mm.tile([P, MM_N], FP32, tag="pm")
                nc.tensor.matmul(
                    pm,
                    lhsT=XgT[:, n * P : (n + 1) * P],
                    rhs=YnT[:, m * MM_N : (m + 1) * MM_N],
                    start=True,
                    stop=True,
                )
                evac(ob[:, m * MM_N : (m + 1) * MM_N], pm, scale=xinv[:, n : n + 1])
            nc.sync.dma_start(out=OUT[n * P : (n + 1) * P, :], in_=ob)

    nc.compile()
    return nc


def _get_program():
    global _CACHED_NC
    if _CACHED_NC is None:
        _CACHED_NC = _build_program()
    return _CACHED_NC


def kernel(X, Y, A_1, A_2, _trace=False, _trace_kwargs=None):
    X = np.asarray(X, dtype=np.float32)
    Y = np.asarray(Y, dtype=np.float32)
    A_1 = np.asarray(A_1, dtype=np.float32)
    A_2 = np.asarray(A_2, dtype=np.float32)
    assert X.shape == (B, N, D), X.shape

    nc = _get_program()
    in_maps = [
        {
            "X": np.ascontiguousarray(X[b]),
            "Y": np.ascontiguousarray(Y[b]),
            "A_1": np.ascontiguousarray(A_1[b]),
            "A_2": np.ascontiguousarray(A_2[b]),
        }
        for b in range(B)
    ]
    res = run_bass_kernel_spmd(
        nc,
        in_maps,
        list(range(B)),
        trace=_trace,
        **(_trace_kwargs or {}),
    )
    out = np.stack([res.results[b]["out"] for b in range(B)], axis=0)
    if _trace:
        return out, res
    return out


# revision 3
# speedup vs baseline: 1.0515x; 1.0515x over previous
"""Gated cosine-affinity kernel for Trainium2 (Bass/Tile), 8-core SPMD.

Problem: for each batch b (B=8):
    Xg = A_1 * X;  Yg = A_2 * Y            (elementwise gates)
    out[b] = normalize_rows(Xg) @ normalize_rows(Yg).T      (2048 x 2048)
with row norm = sqrt(max(|row|^2, 1e-6)).

Sharding: data-parallel over batch — one batch element per NeuronCore.
Per-core pipeline:
  stage 1: gate X/Y, compute row norms (ACT Square+accum, Newton-refined
           sqrt), normalize Y, PE-transpose both into d-major layout.
  stage 2: 16x4 matmuls (d=128 contraction on partitions), fold X's
           1/norm into the PSUM->SBUF evacuation as a per-partition
           scale, stream 1 MiB row-block stores to DRAM.

The kernel is memory-bound: ~21 MB of HBM traffic per core (4 MiB loads
+ 16 MiB stores) against ~360 GB/s per-core HBM bandwidth.
"""

import numpy as np
from contextlib import ExitStack

import concourse.bass as bass
import concourse.tile as tile
from concourse import bacc, mybir
from concourse.bass_utils import run_bass_kernel_spmd
from concourse.masks import make_identity

B = 8
N = 2048          # rows of X (output rows)
M = 2048          # rows of Y (output cols)
D = 128           # feature dim == partition count == contraction dim
P = 128
EPS = 1e-6
NCH = N // P      # 16 row-chunks per tensor
MM_N = 512        # matmul moving free dim (one PSUM bank of fp32)
NMM = M // MM_N   # 4 matmuls per output row-block

FP32 = mybir.dt.float32
AF = mybir.ActivationFunctionType

_CACHED_NC = None


def _build_program():
    nc = bacc.Bacc("TRN2", target_bir_lowering=False, debug=False, num_devices=B)

    Xd = nc.dram_tensor("X", [N, D], FP32, kind="ExternalInput")
    Yd = nc.dram_tensor("Y", [M, D], FP32, kind="ExternalInput")
    A1d = nc.dram_tensor("A_1", [N, D], FP32, kind="ExternalInput")
    A2d = nc.dram_tensor("A_2", [M, D], FP32, kind="ExternalInput")
    OUT = nc.dram_tensor("out", [N, M], FP32, kind="ExternalOutput")

    with tile.TileContext(nc) as tc, ExitStack() as ctx:
        consts = ctx.enter_context(tc.tile_pool(name="consts", bufs=1))
        raw = ctx.enter_context(tc.tile_pool(name="raw", bufs=1))
        gated = ctx.enter_context(tc.tile_pool(name="gated", bufs=1))
        small = ctx.enter_context(tc.tile_pool(name="small", bufs=1))
        scratch = ctx.enter_context(tc.tile_pool(name="scratch", bufs=2))
        yn_pool = ctx.enter_context(tc.tile_pool(name="yn", bufs=4))
        tmat = ctx.enter_context(tc.tile_pool(name="tmat", bufs=1))
        ob_pool = ctx.enter_context(tc.tile_pool(name="ob", bufs=3))
        psum_t = ctx.enter_context(tc.tile_pool(name="psum_t", bufs=3, space="PSUM"))
        psum_mm = ctx.enter_context(tc.tile_pool(name="psum_mm", bufs=5, space="PSUM"))

        ident = consts.tile([P, P], FP32)
        make_identity(nc, ident)

        # Alternate PSUM evacuations between VectorE and ScalarE so neither
        # engine becomes the bottleneck (both are ~0.6us per [128,512] tile).
        copy_state = {"i": 0}

        def evac(dst, src, scale=None):
            use_vector = copy_state["i"] % 2 == 0
            copy_state["i"] += 1
            if scale is None:
                if use_vector:
                    nc.vector.tensor_copy(dst, src)
                else:
                    nc.scalar.copy(dst, src)
            else:
                if use_vector:
                    nc.vector.tensor_scalar_mul(dst, src, scale)
                else:
                    nc.scalar.mul(dst, src, scale)

        def rownorm_inv(sums, name):
            """inv = 1/sqrt(max(sums, EPS)), Newton-refined against ACT's
            low-precision Sqrt table. All ops on [128, NCH] — negligible."""
            v = small.tile([P, NCH], FP32, tag=f"{name}_v")
            s = small.tile([P, NCH], FP32, tag=f"{name}_s")
            r = small.tile([P, NCH], FP32, tag=f"{name}_r")
            t = small.tile([P, NCH], FP32, tag=f"{name}_t")
            inv = small.tile([P, NCH], FP32, tag=f"{name}_inv")
            nc.vector.tensor_scalar_max(v, sums, EPS)
            nc.scalar.sqrt(s, v)
            nc.vector.reciprocal(r, s)
            nc.vector.tensor_mul(t, v, r)          # t = v/s
            nc.vector.tensor_add(t, t, s)          # t = s + v/s
            nc.vector.tensor_scalar_mul(t, t, 0.5)  # Newton: sqrt(v) to ~fp32
            nc.vector.reciprocal(inv, t)
            return inv

        def load_and_gate(Td, Ad, name):
            """Load T and its gate, multiply, and accumulate row sum-squares
            (ACT Square w/ accum_out). Returns (gated_tile, sums_tile)."""
            Tv = Td.rearrange("(c p) d -> p c d", p=P)
            Av = Ad.rearrange("(c p) d -> p c d", p=P)
            traw = raw.tile([P, NCH, D], FP32, tag=f"{name}_raw")
            araw = raw.tile([P, NCH, D], FP32, tag=f"{name}_araw")
            nc.sync.dma_start(out=traw, in_=Tv)
            nc.sync.dma_start(out=araw, in_=Av)
            g = gated.tile([P, NCH, D], FP32, tag=f"{name}_g")
            sums = small.tile([P, NCH], FP32, tag=f"{name}_sums")
            for c in range(NCH):
                nc.vector.tensor_mul(g[:, c, :], traw[:, c, :], araw[:, c, :])
                sq = scratch.tile([P, D], FP32, tag="sq")
                nc.scalar.activation(
                    sq, g[:, c, :], AF.Square, accum_out=sums[:, c : c + 1]
                )
            return g, sums

        # ---- stage 1: Y (normalized pre-transpose; its norm scales output
        # columns, which must be applied before the matmul) ----
        yg, ysums = load_and_gate(Yd, A2d, "y")
        yinv = rownorm_inv(ysums, "y")
        # float32r: 4-byte reduced-mantissa matmul format; streams at
        # 1 row/cycle (vs 4 for fp32). Producers must write it directly.
        FP32R = mybir.dt.float32r
        YnT = tmat.tile([P, M], FP32R, tag="YnT")
        for c in range(NCH):
            yn = yn_pool.tile([P, D], FP32, tag="yn")
            nc.vector.tensor_scalar_mul(yn, yg[:, c, :], yinv[:, c : c + 1])
            pt = psum_t.tile([P, P], FP32, tag="pt")
            nc.tensor.transpose(pt, yn, ident)
            evac(YnT[:, c * P : (c + 1) * P], pt)

        # ---- stage 1: X (transposed unnormalized; 1/norm is folded into the
        # stage-2 PSUM evacuation as a per-partition scale) ----
        xg, xsums = load_and_gate(Xd, A1d, "x")
        XgT = tmat.tile([P, N], FP32R, tag="XgT")
        for c in range(NCH):
            pt = psum_t.tile([P, P], FP32, tag="pt")
            nc.tensor.transpose(pt, xg[:, c, :], ident)
            evac(XgT[:, c * P : (c + 1) * P], pt)
        xinv = rownorm_inv(xsums, "x")

        # ---- stage 2: row-block matmuls + scaled evacuation + 1 MiB stores ----
        for n in range(NCH):
            ob = ob_pool.tile([P, M], FP32, tag="ob")
            for m in range(NMM):
                pm = psum_mm.tile([P, MM_N], FP32, tag="pm")
                nc.tensor.matmul(
                    pm,
                    lhsT=XgT[:, n * P : (n + 1) * P],
                    rhs=YnT[:, m * MM_N : (m + 1) * MM_N],
                    start=True,
                    stop=True,
                )
                evac(ob[:, m * MM_N : (m + 1) * MM_N], pm, scale=xinv[:, n : n + 1])
            nc.sync.dma_start(out=OUT[n * P : (n + 1) * P, :], in_=ob)

    nc.compile()
    return nc


def _get_program():
    global _CACHED_NC
    if _CACHED_NC is None:
        _CACHED_NC = _build_program()
    return _CACHED_NC


def kernel(X, Y, A_1, A_2, _trace=False, _trace_kwargs=None):
    X = np.asarray(X, dtype=np.float32)
    Y = np.asarray(Y, dtype=np.float32)
    A_1 = np.asarray(A_1, dtype=np.float32)
    A_2 = np.asarray(A_2, dtype=np.float32)
    assert X.shape == (B, N, D), X.shape

    nc = _get_program()
    in_maps = [
        {
            "X": np.ascontiguousarray(X[b]),
            "Y": np.ascontiguousarray(Y[b]),
            "A_1": np.ascontiguousarray(A_1[b]),
            "A_2": np.ascontiguousarray(A_2[b]),
        }
        for b in range(B)
    ]
    res = run_bass_kernel_spmd(
        nc,
        in_maps,
        list(range(B)),
        trace=_trace,
        **(_trace_kwargs or {}),
    )
    out = np.stack([res.results[b]["out"] for b in range(B)], axis=0)
    if _trace:
        return out, res
    return out
